# revision 10
# baseline (speedup 1.0000x reference)
"""Trainium2 Bass kernel for linear (taylor/sparse) attention.

Reference computation (per batch b, with xf = x.reshape(b, C, N)):
    Q = Wq@xf + bq            [Cqk, N]
    K = Wk@xf + bk            [Cqk, N]
    V = Wv@xf + bv            [C, N]
    Qh = Q / ||Q||_2 (per position, channel dim)
    Kh = K / ||K||_2
    tailor[n]   = 1 / (N + Qh[:,n] . (sum_n Kh + eps))
    matrix      = Kh @ V^T    [Cqk, C]
    out[:, n]   = gamma * tailor[n] * (sum_n V + matrix^T @ Qh[:,n])

Distribution: 8 cores = 4 batches x 2 halves of N, pairwise AllReduce of the
small factor F = Kh_aug @ [V' | 1]  (V' = gamma*Wv@x, biases folded in later).

Phase-1 structure (per core, 64 n-major tiles of 128 positions):
  - fused [Q|K|V] projection per tile via x-stationary matmuls into PSUM
    pair-tiles ([128, 2, 512] f32, two tiles per PSUM pool buffer)
  - per pair: one TT-add writes biased [Qb|Kb] into the group kv tile (bf16)
  - per tile: one TS-copy evacuates V' to the kv tile (engines alternate V/S)
  - per group of 4 tiles: gpsimd squares [Qb|Kb], one vector tensor_reduce
    produces all 8 per-tile sums of squares, batched sqrt + reciprocal
  - kh_t = Kb_t * rk_t (gpsimd), col 32 of kh is a pre-set ones column
  - factor matmul accumulates F[33, 257] over all tiles
Group tails are software-pipelined (sqrt/recip deferred one group, kh+factor
matmuls deferred two groups) so no engine stream ever stalls on a same-group
dependency.

Phase 1.5: F AllReduce(add) via DRAM; the gap is filled with the channel-major
qx GEMM (Qb rows + ||Qb|| row for phase 2) split across scalar+vector engines.

Phase 2: per tile, psum2[128, 257] = qx_tile^T @ Mx;  out = psum2[:,0:256] *
recip(psum2[:,256]), written bf16 n-major; host transposes/casts.
"""

import ml_dtypes
import numpy as np
from contextlib import ExitStack

import concourse.bass as bass
import concourse.bacc as bacc
import concourse.tile as tile
from concourse import mybir
from concourse import bass_utils
from concourse.masks import make_identity

F32 = mybir.dt.float32
BF16 = mybir.dt.bfloat16
ALU = mybir.AluOpType
ACTF = mybir.ActivationFunctionType

B, C, HH, WW = 4, 256, 128, 128
N = HH * WW            # 16384 positions per batch
NSH = N // 2           # 8192 positions per core
CQK = 32
WID = 2 * CQK + C      # 320: [Q | K | V] fused projection width
FD = C + 1             # 257: [V | 1] rhs / factor / Mx free width
KVW = 360              # kv row stride: [Qb|Kb|V|1|Kh|1|pad]
NT512 = NSH // 512     # 16 x-chunks == 16 groups of 4 tiles
NT128 = NSH // 128     # 64 tiles
EPS = 1e-6

_CACHE = {}


def _build():
    nc = bacc.Bacc("TRN2", target_bir_lowering=False, debug=False, num_devices=8)

    xs = nc.dram_tensor("xs", [C, NSH], BF16, kind="ExternalInput").ap()
    wcat = nc.dram_tensor("wcat", [C, WID], BF16, kind="ExternalInput").ap()
    bias2 = nc.dram_tensor("bias2", [2 * 2 * CQK], F32, kind="ExternalInput").ap()
    bq_in = nc.dram_tensor("bq", [CQK, 1], F32, kind="ExternalInput").ap()
    bvg = nc.dram_tensor("bvg", [C], F32, kind="ExternalInput").ap()
    out = nc.dram_tensor("out", [NSH, C], BF16, kind="ExternalOutput").ap()

    with tile.TileContext(nc) as tc, ExitStack() as ctx:
        _body(ctx, tc, nc, xs, wcat, bias2, bq_in, bvg, out)

    nc.compile()
    return nc


def _body(ctx, tc, nc, xs, wcat, bias2, bq_in, bvg, out):
    singles = ctx.enter_context(tc.tile_pool(name="singles", bufs=1))
    xpool = ctx.enter_context(tc.tile_pool(name="x", bufs=1))
    kvpool = ctx.enter_context(tc.tile_pool(name="kv", bufs=3))
    sqpool = ctx.enter_context(tc.tile_pool(name="sq", bufs=2))
    smalls = ctx.enter_context(tc.tile_pool(name="smalls", bufs=8))
    outpool = ctx.enter_context(tc.tile_pool(name="outp", bufs=3))

    ps_proj = ctx.enter_context(tc.tile_pool(name="ps_proj", bufs=3, space="PSUM"))
    ps_fac = ctx.enter_context(tc.tile_pool(name="ps_fac", bufs=1, space="PSUM"))
    dram = ctx.enter_context(tc.tile_pool(name="dram", bufs=1, space="DRAM"))

    # ---- one-time setup ----
    wcat_sb = singles.tile([128, 2, WID], BF16)
    nc.sync.dma_start(wcat_sb[:], wcat.rearrange("(cb cp) w -> cp cb w", cb=2))
    bias2_rep = singles.tile([128, 2, 2 * CQK], F32)  # [bq|bk] twice, bcast 128p
    nc.gpsimd.dma_start(
        bias2_rep[:],
        bias2.rearrange("(r w) -> r w", r=2).unsqueeze(0).partition_broadcast(128),
    )
    bq_col = singles.tile([CQK, 1], F32)
    nc.gpsimd.dma_start(bq_col[:], bq_in)
    bv_rep = singles.tile([CQK + 1, C], F32)
    nc.gpsimd.dma_start(
        bv_rep[:], bvg.unsqueeze(0).partition_broadcast(CQK + 1).squeeze(1)
    )
    ident = singles.tile([128, 128], F32)
    make_identity(nc, ident[:])

    qx = singles.tile([CQK + 1, NSH], BF16)         # channel-major Qb + ||Qb|| row
    ssqk = singles.tile([128, 2 * NT128], F32)      # [ssq_t, ssk_t] col pairs
    psf = ps_fac.tile([CQK + 1, 2, 512], F32, tag="fac")  # factor accum (2 banks)

    # persistent kv group tiles (ones col pre-set) and kh tiles
    kv_tiles = []
    for i in range(3):
        kvt = kvpool.tile([128, 4, KVW], BF16, tag=f"kv{i}", bufs=1, name=f"kv{i}")
        nc.gpsimd.memset(kvt[:, :, WID : WID + 1], 1.0)
        nc.gpsimd.memset(kvt[:, :, WID + 33 : WID + 34], 1.0)
        kv_tiles.append(kvt)

    # PE warm-up fodder (no DMA dependency)
    warm = singles.tile([128, 512], BF16)
    nc.vector.memset(warm[:], 0.125)
    # CC warm-up: a tiny early AllReduce absorbs the ~10us collective firmware
    # startup so the real factor AllReduce starts hot
    ccw_loc = singles.tile([1, 8], F32)
    nc.vector.memset(ccw_loc[:], 0.0)
    ccw_in = dram.tile([1, 8], F32, tag="ccw", name="ccw_in")
    ccw_out = dram.tile([1, 8], F32, tag="ccw", name="ccw_out")
    nc.sync.dma_start(ccw_in[:], ccw_loc[:])
    nc.gpsimd.collective_compute(
        "AllReduce",
        ALU.add,
        replica_groups=[[0, 1], [2, 3], [4, 5], [6, 7]],
        ins=[ccw_in.opt()],
        outs=[ccw_out.opt()],
    )

    xt_tiles = [None] * NT512
    pair_tiles = [None] * (2 * NT512)
    sk_tiles = [None] * NT512
    rk_tiles = [None] * NT512

    def tail_a(g):
        # sqrt + recip of the 4 per-tile ssk values, then Kh = Kb * rk for the
        # whole group in one broadcast tensor_tensor
        kv4 = kv_tiles[g % 3]
        sk4 = smalls.tile([128, 4], F32, tag="sm", name=f"sk{g}")
        rk4 = smalls.tile([128, 4], F32, tag="sm", name=f"rk{g}")
        nc.scalar.sqrt(sk4[:], ssqk[:, 8 * g + 1 : 8 * g + 8 : 2])
        nc.vector.reciprocal(rk4[:], sk4[:])
        nc.gpsimd.tensor_tensor(
            kv4[:, :, WID + 1 : WID + 33],
            kv4[:, :, CQK : 2 * CQK],
            rk4.unsqueeze(2).broadcast_to([128, 4, CQK]),
            ALU.mult,
        )
        sk_tiles[g] = sk4
        rk_tiles[g] = rk4

    def tail_b(g):
        # factor matmuls for group g (lhsT = [Kh | 1] view of the kv tile);
        # even/odd tiles alternate psum banks so back-to-back matmuls never
        # pay the same-bank drain penalty
        kv4 = kv_tiles[g % 3]
        for u in range(4):
            t = 4 * g + u
            nc.tensor.matmul(
                psf[:, t % 2, 0:FD], kv4[:, u, WID + 1 : WID + 34],
                kv4[:, u, 2 * CQK : WID + 1],
                start=(t < 2), stop=(t >= NT128 - 2),
            )

    # ---- phase 1 ----
    for w in range(8):
        pw = ps_proj.tile([128, 2, 512], F32, tag="proj", name=f"warm{w}")
        nc.tensor.matmul(pw[:, 0, :], warm[:, 0:128], warm[:], start=True, stop=True)

    for j in range(NT512):
        xt = xpool.tile([128, 2, 512], BF16, tag=f"xt{j}", name=f"xt{j}")
        eng = nc.sync if j % 2 == 0 else nc.scalar
        eng.dma_start(
            xt[:],
            xs.rearrange("(cb cp) n -> cp cb n", cb=2)[:, :, j * 512 : (j + 1) * 512],
        )
        xt_tiles[j] = xt

        kv4 = kv_tiles[j % 3]
        pairs = []
        for h in range(2):
            pt = ps_proj.tile([128, 2, 512], F32, tag="proj", name=f"proj{j}_{h}")
            pairs.append(pt)
            pair_tiles[2 * j + h] = pt
            for cb in range(2):
                for s in range(2):
                    u = 2 * h + s
                    nc.tensor.matmul(
                        pt[:, s, 0:WID],
                        xt[:, cb, u * 128 : (u + 1) * 128],
                        wcat_sb[:, cb, :],
                        start=(cb == 0), stop=(cb == 1),
                    )
            # V' evacuation: both tiles of the pair in one strided op
            nc.scalar.copy(
                kv4[:, 2 * h : 2 * h + 2, 2 * CQK : WID],
                pt[:, :, 2 * CQK : WID],
            )
            # biased [Qb|Kb] for the pair
            nc.vector.tensor_tensor(
                kv4[:, 2 * h : 2 * h + 2, 0 : 2 * CQK],
                pt[:, :, 0 : 2 * CQK],
                bias2_rep[:],
                ALU.add,
            )
        # squares + per-tile reductions
        sq4 = sqpool.tile([128, 4, 2 * CQK], BF16, tag="sq", name=f"sq{j}")
        nc.gpsimd.tensor_tensor(
            sq4[:], kv4[:, :, 0 : 2 * CQK], kv4[:, :, 0 : 2 * CQK], ALU.mult
        )
        nc.vector.tensor_reduce(
            ssqk[:, 8 * j : 8 * j + 8],
            sq4.rearrange("p a (r c) -> p (a r) c", c=CQK),
            mybir.AxisListType.X,
            ALU.add,
        )
        # deferred tails
        if j >= 1:
            tail_a(j - 1)
        if j >= 2:
            tail_b(j - 2)
    tail_a(NT512 - 1)
    tail_b(NT512 - 2)
    tail_b(NT512 - 1)

    # ---- phase 1.5: factor AllReduce + gap work ----
    fac_loc = singles.tile([CQK + 1, FD], F32)
    fac_tmp = singles.tile([CQK + 1, FD], F32)
    nc.scalar.copy(fac_tmp[:], psf[:, 0, 0:FD])
    nc.vector.tensor_tensor(fac_loc[:], psf[:, 1, 0:FD], fac_tmp[:], ALU.add)
    cc_in = dram.tile([CQK + 1, FD], F32)
    cc_out = dram.tile([CQK + 1, FD], F32)
    nc.scalar.dma_start(cc_in[:], fac_loc[:])
    nc.gpsimd.collective_compute(
        "AllReduce",
        ALU.add,
        replica_groups=[[0, 1], [2, 3], [4, 5], [6, 7]],
        ins=[cc_in.opt()],
        outs=[cc_out.opt()],
    )
    facg = singles.tile([CQK + 1, FD], F32)
    nc.scalar.dma_start(facg[:], cc_out[:])

    # gap work: channel-major qx rows (Qb) split across scalar+vector engines
    for j0 in range(0, NT512, 2):
        psqs = []
        for j in (j0, j0 + 1):
            psqs.append(ps_proj.tile([CQK, 512], F32, tag="proj", name=f"psq{j}"))
        for cb in range(2):
            for i, j in enumerate((j0, j0 + 1)):
                nc.tensor.matmul(
                    psqs[i][:], wcat_sb[:, cb, 0:CQK], xt_tiles[j][:, cb, :],
                    start=(cb == 0), stop=(cb == 1),
                )
        for i, j in enumerate((j0, j0 + 1)):
            dst = qx[0:CQK, j * 512 : (j + 1) * 512]
            if j % 2 == 0:
                nc.scalar.activation(
                    dst, psqs[i][:], ACTF.Identity, bias=bq_col[:], scale=1.0
                )
            else:
                nc.vector.tensor_scalar_add(dst, psqs[i][:], bq_col[:])

    # gap warm-keeper: f32 matmuls chained on fac_loc keep the PE HAM-warm
    # across the collective so phase 2 starts at full clock
    for w in range(14):
        pwk = ps_fac.tile([128, 2, 512], F32, tag="fac", name=f"wk{w}")
        nc.tensor.matmul(
            pwk[:, 0, 0:FD], fac_loc[:, 0:128], fac_loc[:], start=True, stop=True
        )

    # ||Qb|| row: batched sqrt, PE transpose, DRAM bounce into one row
    normq = singles.tile([128, NT128], F32)
    nc.scalar.sqrt(normq[:], ssqk[:, 0 : 2 * NT128 : 2])
    pst = ps_fac.tile([NT128, 128], F32, tag="fac", name="pst")
    nc.tensor.transpose(pst[:], normq[:], ident[:])
    trT = singles.tile([NT128, 128], BF16)
    nc.vector.tensor_copy(trT[:], pst[:])
    row_scratch = dram.tile([NT128, 128], BF16)
    nc.sync.dma_start(row_scratch[:], trT[:])
    nc.sync.dma_start(
        qx[CQK : CQK + 1, :],
        row_scratch[:].rearrange("a b -> (a b)").unsqueeze(0),
    )

    # ---- Mx build [33, 257] ----
    mx = singles.tile([CQK + 1, FD], BF16)
    tmp32 = singles.tile([CQK, C], F32)
    nc.vector.tensor_scalar_mul(tmp32[:], bv_rep[0:CQK, :], facg[0:CQK, C : C + 1])
    nc.vector.tensor_tensor(mx[0:CQK, 0:C], tmp32[:], facg[0:CQK, 0:C], ALU.add)
    nc.vector.scalar_tensor_tensor(
        mx[CQK : CQK + 1, 0:C], bv_rep[CQK : CQK + 1, :], float(N),
        facg[CQK : CQK + 1, 0:C], ALU.mult, ALU.add,
    )
    nc.vector.tensor_scalar_add(mx[:, C:FD], facg[:, C:FD], EPS)

    # ---- phase 2 ----
    out4 = out.rearrange("(t4 u p) c -> t4 p u c", u=4, p=128)
    for t4 in range(NT128 // 4):
        ot = outpool.tile([128, 4, C], BF16, tag="ot", name=f"ot{t4}")
        rc_pair = [None, None]
        for h in range(2):
            if (2 * t4 + h) % 4 == 3:
                pt = ps_fac.tile([128, 2, 512], F32, tag="fac",
                                 name=f"ps2_{t4}_{h}")
            else:
                pt = ps_proj.tile([128, 2, 512], F32, tag="proj", name=f"ps2_{t4}_{h}")
            for s in range(2):
                t = 4 * t4 + 2 * h + s
                nc.tensor.matmul(
                    pt[:, s, 0:FD], qx[:, t * 128 : (t + 1) * 128], mx[:],
                    start=True, stop=True,
                )
            rc = smalls.tile([128, 2], F32, tag="sm", name=f"rc{t4}_{h}")
            nc.vector.reciprocal(rc[:], pt[:, :, C : C + 1].rearrange("p a b -> p (a b)"))
            rc_pair[h] = (pt, rc)
        pt, rc = rc_pair[0]
        nc.vector.scalar_tensor_tensor(
            ot[:, 0:2, :], pt[:, :, 0:C], 1.0,
            rc.unsqueeze(2).broadcast_to([128, 2, C]), ALU.mult, ALU.mult,
        )
        pt, rc = rc_pair[1]
        for s in range(2):
            nc.scalar.mul(ot[:, 2 + s, :], pt[:, s, 0:C], rc[:, s : s + 1])
        nc.sync.dma_start(out4[t4], ot[:])


def _get_nc():
    if "nc" not in _CACHE:
        _CACHE["nc"] = _build()
    return _CACHE["nc"]


def _prep_in_maps(x, Wq, bq, Wk, bk, Wv, bv, gamma):
    g = float(np.asarray(gamma).reshape(-1)[0])
    wcat = np.concatenate(
        [
            Wq.T.astype(np.float32),
            Wk.T.astype(np.float32),
            (g * Wv).T.astype(np.float32),
        ],
        axis=1,
    ).astype(ml_dtypes.bfloat16)
    wcat = np.ascontiguousarray(wcat)
    bias1 = np.concatenate([bq.astype(np.float32), bk.astype(np.float32)])
    bias2 = np.concatenate([bias1, bias1])
    bvg = np.ascontiguousarray(g * bv, dtype=np.float32)
    bq_col = np.ascontiguousarray(bq.reshape(CQK, 1), dtype=np.float32)

    xf = np.asarray(x, dtype=np.float32).reshape(B, C, N)
    in_maps = []
    for core in range(8):
        b, h = core // 2, core % 2
        xsh = np.ascontiguousarray(
            xf[b, :, h * NSH : (h + 1) * NSH].astype(ml_dtypes.bfloat16)
        )
        in_maps.append(
            {
                "xs": xsh,
                "wcat": wcat,
                "bias2": bias2,
                "bq": bq_col,
                "bvg": bvg,
            }
        )
    return in_maps


def run(inputs, trace=False):
    nc = _get_nc()
    in_maps = _prep_in_maps(**inputs)
    res = bass_utils.run_bass_kernel_spmd(
        nc, in_maps, core_ids=list(range(8)), trace=trace
    )
    outf = np.empty((B, C, N), np.float32)
    for core in range(8):
        b, h = core // 2, core % 2
        outf[b, :, h * NSH : (h + 1) * NSH] = (
            res.results[core]["out"].astype(np.float32).T
        )
    return outf.reshape(B, C, HH, WW), res


def kernel(**inputs):
    out, _ = run(inputs, trace=False)
    return out


# revision 11
# speedup vs baseline: 1.0816x; 1.0816x over previous
"""Trainium2 Bass kernel for linear (taylor/sparse) attention.

Reference computation (per batch b, with xf = x.reshape(b, C, N)):
    Q = Wq@xf + bq            [Cqk, N]
    K = Wk@xf + bk            [Cqk, N]
    V = Wv@xf + bv            [C, N]
    Qh = Q / ||Q||_2 (per position, channel dim)
    Kh = K / ||K||_2
    tailor[n]   = 1 / (N + Qh[:,n] . (sum_n Kh + eps))
    matrix      = Kh @ V^T    [Cqk, C]
    out[:, n]   = gamma * tailor[n] * (sum_n V + matrix^T @ Qh[:,n])

Distribution: 8 cores = 4 batches x 2 halves of N, pairwise AllReduce of the
small factor F = Kh_aug @ [V' | 1]  (V' = gamma*Wv@x, biases folded in later).

Phase-1 structure (per core, 64 n-major tiles of 128 positions):
  - fused [Q|K|V] projection per tile via x-stationary matmuls into PSUM
    pair-tiles ([128, 2, 512] f32, two tiles per PSUM pool buffer)
  - per pair: one TT-add writes biased [Qb|Kb] into the group kv tile (bf16)
  - per tile: one TS-copy evacuates V' to the kv tile (engines alternate V/S)
  - per group of 4 tiles: gpsimd squares [Qb|Kb], one vector tensor_reduce
    produces all 8 per-tile sums of squares, batched sqrt + reciprocal
  - kh_t = Kb_t * rk_t (gpsimd), col 32 of kh is a pre-set ones column
  - factor matmul accumulates F[33, 257] over all tiles
Group tails are software-pipelined (sqrt/recip deferred one group, kh+factor
matmuls deferred two groups) so no engine stream ever stalls on a same-group
dependency.

Phase 1.5: F AllReduce(add) via DRAM; the gap is filled with the channel-major
qx GEMM (Qb rows + ||Qb|| row for phase 2) split across scalar+vector engines.

Phase 2: per tile, psum2[128, 257] = qx_tile^T @ Mx;  out = psum2[:,0:256] *
recip(psum2[:,256]), written bf16 n-major; host transposes/casts.
"""

import ml_dtypes
import numpy as np
from contextlib import ExitStack

import concourse.bass as bass
import concourse.bacc as bacc
import concourse.tile as tile
from concourse import mybir
from concourse import bass_utils
from concourse.masks import make_identity

F32 = mybir.dt.float32
BF16 = mybir.dt.bfloat16
ALU = mybir.AluOpType
ACTF = mybir.ActivationFunctionType

B, C, HH, WW = 4, 256, 128, 128
N = HH * WW            # 16384 positions per batch
NSH = N // 2           # 8192 positions per core
CQK = 32
WID = 2 * CQK + C      # 320: [Q | K | V] fused projection width
FD = C + 1             # 257: [V | 1] rhs / factor / Mx free width
KVW = 360              # kv row stride: [Qb|Kb|V|1|Kh|1|pad]
NT512 = NSH // 512     # 16 x-chunks == 16 groups of 4 tiles
NT128 = NSH // 128     # 64 tiles
EPS = 1e-6

_CACHE = {}


def _build():
    nc = bacc.Bacc("TRN2", target_bir_lowering=False, debug=False, num_devices=8)

    xs = nc.dram_tensor("xs", [C, NSH], BF16, kind="ExternalInput").ap()
    wcat = nc.dram_tensor("wcat", [C, WID], BF16, kind="ExternalInput").ap()
    bias2 = nc.dram_tensor("bias2", [2 * 2 * CQK], F32, kind="ExternalInput").ap()
    bq_in = nc.dram_tensor("bq", [CQK, 1], F32, kind="ExternalInput").ap()
    bvg = nc.dram_tensor("bvg", [C], F32, kind="ExternalInput").ap()
    out = nc.dram_tensor("out", [NSH, C], BF16, kind="ExternalOutput").ap()

    with tile.TileContext(nc) as tc, ExitStack() as ctx:
        _body(ctx, tc, nc, xs, wcat, bias2, bq_in, bvg, out)

    nc.compile()
    return nc


def _body(ctx, tc, nc, xs, wcat, bias2, bq_in, bvg, out):
    singles = ctx.enter_context(tc.tile_pool(name="singles", bufs=1))
    xpool = ctx.enter_context(tc.tile_pool(name="x", bufs=1))
    kvpool = ctx.enter_context(tc.tile_pool(name="kv", bufs=3))
    sqpool = ctx.enter_context(tc.tile_pool(name="sq", bufs=2))
    smalls = ctx.enter_context(tc.tile_pool(name="smalls", bufs=8))
    outpool = ctx.enter_context(tc.tile_pool(name="outp", bufs=3))

    ps_proj = ctx.enter_context(tc.tile_pool(name="ps_proj", bufs=3, space="PSUM"))
    ps_fac = ctx.enter_context(tc.tile_pool(name="ps_fac", bufs=1, space="PSUM"))
    dram = ctx.enter_context(tc.tile_pool(name="dram", bufs=1, space="DRAM"))

    # ---- one-time setup ----
    wcat_sb = singles.tile([128, 2, WID], BF16)
    nc.sync.dma_start(wcat_sb[:], wcat.rearrange("(cb cp) w -> cp cb w", cb=2))
    bias2_rep = singles.tile([128, 2, 2 * CQK], F32)  # [bq|bk] twice, bcast 128p
    nc.gpsimd.dma_start(
        bias2_rep[:],
        bias2.rearrange("(r w) -> r w", r=2).unsqueeze(0).partition_broadcast(128),
    )
    bq_col = singles.tile([CQK, 1], F32)
    nc.gpsimd.dma_start(bq_col[:], bq_in)
    bv_rep = singles.tile([CQK + 1, C], F32)
    nc.gpsimd.dma_start(
        bv_rep[:], bvg.unsqueeze(0).partition_broadcast(CQK + 1).squeeze(1)
    )
    ident = singles.tile([128, 128], F32)
    make_identity(nc, ident[:])

    qx = singles.tile([CQK + 1, NSH], BF16)         # channel-major Qb + ||Qb|| row
    ssqk = singles.tile([128, 2 * NT128], F32)      # [ssq_t, ssk_t] col pairs
    psf = ps_fac.tile([CQK + 1, 2, 512], F32, tag="fac")  # factor accum (2 banks)

    # persistent kv group tiles (ones col pre-set) and kh tiles
    kv_tiles = []
    for i in range(3):
        kvt = kvpool.tile([128, 4, KVW], BF16, tag=f"kv{i}", bufs=1, name=f"kv{i}")
        nc.gpsimd.memset(kvt[:, :, WID : WID + 1], 1.0)
        nc.gpsimd.memset(kvt[:, :, WID + 33 : WID + 34], 1.0)
        kv_tiles.append(kvt)

    # PE warm-up fodder (no DMA dependency)
    warm = singles.tile([128, 512], BF16)
    nc.vector.memset(warm[:], 0.125)
    # CC warm-up: a tiny early AllReduce absorbs the ~10us collective firmware
    # startup so the real factor AllReduce starts hot
    ccw_loc = singles.tile([1, 8], F32)
    nc.vector.memset(ccw_loc[:], 0.0)
    ccw_in = dram.tile([1, 8], F32, tag="ccw", name="ccw_in")
    ccw_out = dram.tile([1, 8], F32, tag="ccw", name="ccw_out")
    nc.sync.dma_start(ccw_in[:], ccw_loc[:])
    nc.gpsimd.collective_compute(
        "AllReduce",
        ALU.add,
        replica_groups=[[0, 1], [2, 3], [4, 5], [6, 7]],
        ins=[ccw_in.opt()],
        outs=[ccw_out.opt()],
    )

    xt_tiles = [None] * NT512
    pair_tiles = [None] * (2 * NT512)
    sk_tiles = [None] * NT512
    rk_tiles = [None] * NT512

    def tail_a(g):
        # sqrt + recip of the 4 per-tile ssk values, then Kh = Kb * rk for the
        # whole group in one broadcast tensor_tensor
        kv4 = kv_tiles[g % 3]
        sk4 = smalls.tile([128, 4], F32, tag="sm", name=f"sk{g}")
        rk4 = smalls.tile([128, 4], F32, tag="sm", name=f"rk{g}")
        nc.scalar.sqrt(sk4[:], ssqk[:, 8 * g + 1 : 8 * g + 8 : 2])
        nc.vector.reciprocal(rk4[:], sk4[:])
        nc.gpsimd.tensor_tensor(
            kv4[:, :, WID + 1 : WID + 33],
            kv4[:, :, CQK : 2 * CQK],
            rk4.unsqueeze(2).broadcast_to([128, 4, CQK]),
            ALU.mult,
        )
        sk_tiles[g] = sk4
        rk_tiles[g] = rk4

    def tail_b(g):
        # factor matmuls for group g (lhsT = [Kh | 1] view of the kv tile);
        # even/odd tiles alternate psum banks so back-to-back matmuls never
        # pay the same-bank drain penalty
        kv4 = kv_tiles[g % 3]
        for u in range(4):
            t = 4 * g + u
            nc.tensor.matmul(
                psf[:, t % 2, 0:FD], kv4[:, u, WID + 1 : WID + 34],
                kv4[:, u, 2 * CQK : WID + 1],
                start=(t < 2), stop=(t >= NT128 - 2),
            )

    # ---- phase 1 ----
    for w in range(8):
        pw = ps_proj.tile([128, 2, 512], F32, tag="proj", name=f"warm{w}")
        nc.tensor.matmul(pw[:, 0, :], warm[:, 0:128], warm[:], start=True, stop=True)

    for j in range(NT512):
        xt = xpool.tile([128, 2, 512], BF16, tag=f"xt{j}", name=f"xt{j}")
        eng = nc.sync if j % 2 == 0 else nc.scalar
        eng.dma_start(
            xt[:],
            xs.rearrange("(cb cp) n -> cp cb n", cb=2)[:, :, j * 512 : (j + 1) * 512],
        )
        xt_tiles[j] = xt

        kv4 = kv_tiles[j % 3]
        pairs = []
        for h in range(2):
            pt = ps_proj.tile([128, 2, 512], F32, tag="proj", name=f"proj{j}_{h}")
            pairs.append(pt)
            pair_tiles[2 * j + h] = pt
            for cb in range(2):
                for s in range(2):
                    u = 2 * h + s
                    nc.tensor.matmul(
                        pt[:, s, 0:WID],
                        xt[:, cb, u * 128 : (u + 1) * 128],
                        wcat_sb[:, cb, :],
                        start=(cb == 0), stop=(cb == 1),
                    )
            # V' evacuation: both tiles of the pair in one strided op
            nc.scalar.copy(
                kv4[:, 2 * h : 2 * h + 2, 2 * CQK : WID],
                pt[:, :, 2 * CQK : WID],
            )
            # biased [Qb|Kb] for the pair
            nc.vector.tensor_tensor(
                kv4[:, 2 * h : 2 * h + 2, 0 : 2 * CQK],
                pt[:, :, 0 : 2 * CQK],
                bias2_rep[:],
                ALU.add,
            )
        # squares + per-tile reductions
        sq4 = sqpool.tile([128, 4, 2 * CQK], BF16, tag="sq", name=f"sq{j}")
        nc.gpsimd.tensor_tensor(
            sq4[:], kv4[:, :, 0 : 2 * CQK], kv4[:, :, 0 : 2 * CQK], ALU.mult
        )
        nc.vector.tensor_reduce(
            ssqk[:, 8 * j : 8 * j + 8],
            sq4.rearrange("p a (r c) -> p (a r) c", c=CQK),
            mybir.AxisListType.X,
            ALU.add,
        )
        # deferred tails
        if j >= 1:
            tail_a(j - 1)
        if j >= 2:
            tail_b(j - 2)
    tail_a(NT512 - 1)
    tail_b(NT512 - 2)
    tail_b(NT512 - 1)

    # ---- phase 1.5: factor AllReduce + gap work ----
    fac_loc = singles.tile([CQK + 1, FD], F32)
    fac_tmp = singles.tile([CQK + 1, FD], F32)
    nc.scalar.copy(fac_tmp[:], psf[:, 0, 0:FD])
    nc.vector.tensor_tensor(fac_loc[:], psf[:, 1, 0:FD], fac_tmp[:], ALU.add)
    cc_in = dram.tile([CQK + 1, FD], F32)
    cc_out = dram.tile([CQK + 1, FD], F32)
    nc.scalar.dma_start(cc_in[:], fac_loc[:])
    nc.gpsimd.collective_compute(
        "AllReduce",
        ALU.add,
        replica_groups=[[0, 1], [2, 3], [4, 5], [6, 7]],
        ins=[cc_in.opt()],
        outs=[cc_out.opt()],
    )
    facg = singles.tile([CQK + 1, FD], F32)
    nc.scalar.dma_start(facg[:], cc_out[:])

    # gap work: channel-major qx rows (Qb) split across scalar+vector engines.
    # The stationary weights are rebuilt from wcat + a zero derived from the
    # last projection psum pair, which pins these matmuls into the collective
    # gap (the scheduler cannot hoist them into the PE-bound phase 1).
    zq = singles.tile([128, 2, CQK], BF16)
    nc.vector.tensor_scalar_mul(zq[:], pair_tiles[31][:, :, 0:CQK], 0.0)
    wq_gap = singles.tile([128, 2, CQK], BF16)
    nc.vector.tensor_tensor(
        wq_gap[:], wcat_sb[:, :, 0:CQK], zq[:], ALU.add
    )
    for j0 in range(0, NT512, 2):
        psqs = []
        for j in (j0, j0 + 1):
            psqs.append(ps_proj.tile([CQK, 512], F32, tag="proj", name=f"psq{j}"))
        for cb in range(2):
            for i, j in enumerate((j0, j0 + 1)):
                nc.tensor.matmul(
                    psqs[i][:], wq_gap[:, cb, :], xt_tiles[j][:, cb, :],
                    start=(cb == 0), stop=(cb == 1),
                )
        for i, j in enumerate((j0, j0 + 1)):
            dst = qx[0:CQK, j * 512 : (j + 1) * 512]
            if j % 2 == 0:
                nc.scalar.activation(
                    dst, psqs[i][:], ACTF.Identity, bias=bq_col[:], scale=1.0
                )
            else:
                nc.vector.tensor_scalar_add(dst, psqs[i][:], bq_col[:])

    # ||Qb|| row: batched sqrt, PE transpose, DRAM bounce into one row
    normq = singles.tile([128, NT128], F32)
    nc.scalar.sqrt(normq[:], ssqk[:, 0 : 2 * NT128 : 2])
    pst = ps_fac.tile([NT128, 128], F32, tag="fac", name="pst")
    nc.tensor.transpose(pst[:], normq[:], ident[:])
    trT = singles.tile([NT128, 128], BF16)
    nc.vector.tensor_copy(trT[:], pst[:])
    row_scratch = dram.tile([NT128, 128], BF16)
    nc.sync.dma_start(row_scratch[:], trT[:])
    nc.sync.dma_start(
        qx[CQK : CQK + 1, :],
        row_scratch[:].rearrange("a b -> (a b)").unsqueeze(0),
    )

    # ---- Mx build [33, 257] ----
    mx = singles.tile([CQK + 1, FD], BF16)
    tmp32 = singles.tile([CQK, C], F32)
    nc.vector.tensor_scalar_mul(tmp32[:], bv_rep[0:CQK, :], facg[0:CQK, C : C + 1])
    nc.vector.tensor_tensor(mx[0:CQK, 0:C], tmp32[:], facg[0:CQK, 0:C], ALU.add)
    nc.vector.scalar_tensor_tensor(
        mx[CQK : CQK + 1, 0:C], bv_rep[CQK : CQK + 1, :], float(N),
        facg[CQK : CQK + 1, 0:C], ALU.mult, ALU.add,
    )
    nc.vector.tensor_scalar_add(mx[:, C:FD], facg[:, C:FD], EPS)

    # ---- phase 2 ----
    out4 = out.rearrange("(t4 u p) c -> t4 p u c", u=4, p=128)
    for t4 in range(NT128 // 4):
        ot = outpool.tile([128, 4, C], BF16, tag="ot", name=f"ot{t4}")
        rc_pair = [None, None]
        for h in range(2):
            if (2 * t4 + h) % 4 == 3:
                pt = ps_fac.tile([128, 2, 512], F32, tag="fac",
                                 name=f"ps2_{t4}_{h}")
            else:
                pt = ps_proj.tile([128, 2, 512], F32, tag="proj", name=f"ps2_{t4}_{h}")
            for s in range(2):
                t = 4 * t4 + 2 * h + s
                nc.tensor.matmul(
                    pt[:, s, 0:FD], qx[:, t * 128 : (t + 1) * 128], mx[:],
                    start=True, stop=True,
                )
            rc = smalls.tile([128, 2], F32, tag="sm", name=f"rc{t4}_{h}")
            nc.vector.reciprocal(rc[:], pt[:, :, C : C + 1].rearrange("p a b -> p (a b)"))
            rc_pair[h] = (pt, rc)
        pt, rc = rc_pair[0]
        nc.vector.scalar_tensor_tensor(
            ot[:, 0:2, :], pt[:, :, 0:C], 1.0,
            rc.unsqueeze(2).broadcast_to([128, 2, C]), ALU.mult, ALU.mult,
        )
        pt, rc = rc_pair[1]
        for s in range(2):
            nc.scalar.mul(ot[:, 2 + s, :], pt[:, s, 0:C], rc[:, s : s + 1])
        nc.sync.dma_start(out4[t4], ot[:])


def _get_nc():
    if "nc" not in _CACHE:
        _CACHE["nc"] = _build()
    return _CACHE["nc"]


def _prep_in_maps(x, Wq, bq, Wk, bk, Wv, bv, gamma):
    g = float(np.asarray(gamma).reshape(-1)[0])
    wcat = np.concatenate(
        [
            Wq.T.astype(np.float32),
            Wk.T.astype(np.float32),
            (g * Wv).T.astype(np.float32),
        ],
        axis=1,
    ).astype(ml_dtypes.bfloat16)
    wcat = np.ascontiguousarray(wcat)
    bias1 = np.concatenate([bq.astype(np.float32), bk.astype(np.float32)])
    bias2 = np.concatenate([bias1, bias1])
    bvg = np.ascontiguousarray(g * bv, dtype=np.float32)
    bq_col = np.ascontiguousarray(bq.reshape(CQK, 1), dtype=np.float32)

    xf = np.asarray(x, dtype=np.float32).reshape(B, C, N)
    in_maps = []
    for core in range(8):
        b, h = core // 2, core % 2
        xsh = np.ascontiguousarray(
            xf[b, :, h * NSH : (h + 1) * NSH].astype(ml_dtypes.bfloat16)
        )
        in_maps.append(
            {
                "xs": xsh,
                "wcat": wcat,
                "bias2": bias2,
                "bq": bq_col,
                "bvg": bvg,
            }
        )
    return in_maps


def run(inputs, trace=False):
    nc = _get_nc()
    in_maps = _prep_in_maps(**inputs)
    res = bass_utils.run_bass_kernel_spmd(
        nc, in_maps, core_ids=list(range(8)), trace=trace
    )
    outf = np.empty((B, C, N), np.float32)
    for core in range(8):
        b, h = core // 2, core % 2
        outf[b, :, h * NSH : (h + 1) * NSH] = (
            res.results[core]["out"].astype(np.float32).T
        )
    return outf.reshape(B, C, HH, WW), res


def kernel(**inputs):
    out, _ = run(inputs, trace=False)
    return out
